# revision 1
# baseline (speedup 1.0000x reference)
"""Trainium2 Bass kernel for a sigmoid-scored attention decode step with KV cache.

Reference computation (all fp32):
    q = W_query @ x.T ; k = W_key @ x.T ; v = W_value @ x.T          # [4096, 1]
    K = [K_cache | k] ; V = [V_cache | v]                            # [4096, 8193]
    a = sigmoid((q.T @ K) / 64)                                      # [1, 8193]
    z = V @ a.T                                                      # [4096, 1]

Sharding: rows (output dim) of W_q/W_k/W_v/K_cache/V_cache are split across
8 NeuronCores (512 rows each). Each core computes its q/k/v shard and partial
scores over its 512 rows of K; per-chunk AllReduces (4x ~8KB) combine partials
into full scores on every core; sigmoid + the V-weighted sum are then local
per shard. Host only slices inputs and concatenates the output.

Engine mapping per core:
  - q/k/v matvecs and z = V@a contract along the free dim -> DVE custom-op
    TENSOR_TENSOR_REDUCE against a broadcast vector (the native ISA
    tensor_tensor_reduce faults the exec unit on this runtime).
  - scores q.T K contract along partitions -> PE matmuls, K_cache tiles in
    natural [d, t] layout.
  - the score vector is AllReduced in 4 column chunks so collectives,
    sigmoid, PE rank-1 broadcast (ones x a_chunk -> PSUM) and the z-phase
    DVE reduces pipeline against the V_cache DMA stream instead of
    serializing at the end.
"""

import sys

for _p in ("/opt/trn_rl_repo", "/root/.axon_site/_ro/trn_rl_repo"):
    if _p not in sys.path:
        sys.path.append(_p)

import numpy as np

import concourse.bacc as bacc
import concourse.tile as tile
from concourse import mybir
from concourse.bass_utils import run_bass_kernel_spmd
from concourse.dve_ops import TENSOR_TENSOR_REDUCE

N_CORES = 8
E = 4096          # embedding dim (contraction for q/k/v)
D = 4096          # output dim
T = 8192          # cached timesteps
F32 = mybir.dt.float32


def build(n_cores=N_CORES, e=E, d_sh=D // N_CORES, t=T, kv_f=4096, w_f=4096):
    nd = d_sh // 128             # partition-chunks per core
    nc_t = t // kv_f             # cache column groups (2 at kv_f=4096)
    nj = kv_f // 512             # matmul slices per cache tile
    bps_f = min(2048, kv_f)      # broadcast-PSUM chunk (<=4 banks)
    nh = kv_f // bps_f

    nc = bacc.Bacc("TRN2", target_bir_lowering=False, debug=False,
                   num_devices=n_cores)
    x_d = nc.dram_tensor("x", [1, e], F32, kind="ExternalInput").ap()
    wq_d = nc.dram_tensor("wq", [d_sh, e], F32, kind="ExternalInput").ap()
    wk_d = nc.dram_tensor("wk", [d_sh, e], F32, kind="ExternalInput").ap()
    wv_d = nc.dram_tensor("wv", [d_sh, e], F32, kind="ExternalInput").ap()
    kc_d = nc.dram_tensor("kc", [d_sh, t], F32, kind="ExternalInput").ap()
    vc_d = nc.dram_tensor("vc", [d_sh, t], F32, kind="ExternalInput").ap()
    z_d = nc.dram_tensor("z", [128, nd], F32, kind="ExternalOutput").ap()

    with tile.TileContext(nc) as tc:
        with (
            tc.tile_pool(name="w", bufs=3) as wp,            # x + W tiles
            tc.tile_pool(name="stream", bufs=3) as sp,       # K/V cache tiles
            tc.tile_pool(name="scratch", bufs=2) as scp,     # ttr elementwise outs
            tc.tile_pool(name="keep", bufs=1) as kp,         # persistent tiles
            tc.tile_pool(name="acc", bufs=8) as accp,        # [128,1] accumulators
            tc.tile_pool(name="dram", bufs=1, space="DRAM") as dramp,
        ):
            # --- broadcast x across partitions ---
            x_sb = wp.tile([1, e], F32, tag="w", name="x_sb")
            nc.gpsimd.dma_start(x_sb[:], x_d[:])
            bx = kp.tile([128, e], F32, tag="bx", name="bx")
            nc.gpsimd.partition_broadcast(bx[:], x_sb[:])

            ones_sb = kp.tile([1, 128], F32, tag="ones", name="ones_sb")
            nc.vector.memset(ones_sb[:], 1.0)
            ones_col = kp.tile([128, 1], F32, tag="onesc", name="ones_col")
            nc.vector.memset(ones_col[:], 1.0)
            # pre-warm the sigmoid ACT table so the load is off the critical path
            warm = kp.tile([1, 1], F32, tag="warm", name="warm")
            nc.vector.memset(warm[:], 0.0)
            nc.scalar.activation(warm[:], warm[:],
                                 mybir.ActivationFunctionType.Sigmoid,
                                 scale=1.0 / 64.0)

            # --- q/k/v matvecs: qkv_all[:, nd*w + d] = (W[d-chunk] @ x) ---
            qkv_all = kp.tile([128, 3 * nd], F32, tag="qkv", name="qkv_all")

            def w_matvec(w_dram, col0):
                for d in range(nd):
                    wt = wp.tile([128, w_f], F32, tag="w", name=f"wt{col0}_{d}")
                    nc.sync.dma_start(wt[:], w_dram[128 * d:128 * (d + 1), :])
                    sc = scp.tile([128, w_f], F32, tag="sc", name=f"sc{col0}_{d}")
                    nc.vector._custom_dve(
                        TENSOR_TENSOR_REDUCE, out=sc[:], in0=wt[:], in1=bx[:],
                        s0=0.0, s1=1.0,
                        accum_out=qkv_all[:, col0 + d:col0 + d + 1],
                    )

            w_matvec(wq_d, 0)        # q in cols 0..nd-1

            # --- partial scores per column group; AR_0 fires after group 0 ---
            s_sb = kp.tile([1, t + 8], F32, tag="s", name="s_sb")
            a_sb = s_sb  # AR results land back in the same buffer, chunk-local
            nc.vector.memset(s_sb[0:1, t:t + 8], 0.0)
            g0_len = kv_f
            cc_ins = [dramp.tile([1, g0_len], F32, tag="cc_in0", name="cc_in0"),
                      dramp.tile([1, t - g0_len], F32, tag="cc_in1",
                                 name="cc_in1"),
                      dramp.tile([1, 8], F32, tag="cc_in2", name="cc_in2")]
            cc_outs = [dramp.tile([1, g0_len], F32, tag="cc_out0", name="cc_out0"),
                       dramp.tile([1, t - g0_len], F32, tag="cc_out1",
                                  name="cc_out1"),
                       dramp.tile([1, 8], F32, tag="cc_out2", name="cc_out2")]

            psp_ctx = tc.tile_pool(name="ps", bufs=8, space="PSUM")
            psp = psp_ctx.__enter__()

            def score_group(c):
                pss = [psp.tile([1, 512], F32, tag="ps", name=f"ps{c}_{j}")
                       for j in range(nj)]
                for d in range(nd):
                    kt = sp.tile([128, kv_f], F32, tag="kv", name=f"kt{c}_{d}")
                    nc.sync.dma_start(
                        kt[:], kc_d[128 * d:128 * (d + 1),
                                    kv_f * c:kv_f * (c + 1)])
                    for j in range(nj):
                        nc.tensor.matmul(
                            pss[j][:],
                            lhsT=qkv_all[:, d:d + 1],
                            rhs=kt[:, 512 * j:512 * (j + 1)],
                            start=(d == 0), stop=(d == nd - 1),
                        )
                for j in range(nj):
                    nc.vector.tensor_copy(
                        s_sb[0:1, kv_f * c + 512 * j:kv_f * c + 512 * (j + 1)],
                        pss[j][:])

            score_group(0)
            nc.gpsimd.dma_start(cc_ins[0][:], s_sb[0:1, 0:g0_len])
            nc.gpsimd.collective_compute(
                "AllReduce", mybir.AluOpType.add,
                replica_groups=[list(range(n_cores))],
                ins=[cc_ins[0].opt()], outs=[cc_outs[0].opt()],
            )
            w_matvec(wk_d, nd)       # k in cols nd..2nd-1
            for c in range(1, nc_t):
                score_group(c)

            nc.gpsimd.dma_start(cc_ins[1][:], s_sb[0:1, g0_len:t])
            nc.gpsimd.collective_compute(
                "AllReduce", mybir.AluOpType.add,
                replica_groups=[list(range(n_cores))],
                ins=[cc_ins[1].opt()], outs=[cc_outs[1].opt()],
            )

            # --- appended-column score rides its own tiny AR ---
            qk_el = scp.tile([128, nd], F32, tag="qk_el", name="qk_el")
            qk_part = accp.tile([128, 1], F32, tag="acc", name="qk_part")
            nc.vector._custom_dve(
                TENSOR_TENSOR_REDUCE, out=qk_el[:], in0=qkv_all[:, 0:nd],
                in1=qkv_all[:, nd:2 * nd], s0=0.0, s1=1.0,
                accum_out=qk_part[:],
            )
            qk_ps = psp.tile([1, 512], F32, tag="ps", name="qk_ps")
            nc.tensor.matmul(qk_ps[0:1, 0:1], lhsT=ones_col[:],
                             rhs=qk_part[:], start=True, stop=True)
            nc.vector.tensor_copy(s_sb[0:1, t:t + 1], qk_ps[0:1, 0:1])
            nc.gpsimd.dma_start(cc_ins[2][:], s_sb[0:1, t:t + 8])
            nc.gpsimd.collective_compute(
                "AllReduce", mybir.AluOpType.add,
                replica_groups=[list(range(n_cores))],
                ins=[cc_ins[2].opt()], outs=[cc_outs[2].opt()],
            )
            w_matvec(wv_d, 2 * nd)   # v in cols 2nd..3nd-1
            psp_ctx.__exit__(None, None, None)

            # --- per 2048-subchunk: sigmoid -> PE rank-1 broadcast into PSUM;
            # --- z accumulation: DVE reduce of V tiles against broadcast a ---
            z_final = kp.tile([128, nd], F32, tag="z", name="z_final")
            with tc.tile_pool(name="bps", bufs=2, space="PSUM") as bpsp:
                accs = [None] * nd
                for c in range(nc_t):
                    if c == 0:
                        nc.scalar.dma_start(a_sb[0:1, 0:g0_len], cc_outs[0][:])
                    else:
                        nc.scalar.dma_start(a_sb[0:1, g0_len:t],
                                            cc_outs[1][:])
                    bps_tiles = []
                    for h in range(nh):
                        sub = kv_f * c + bps_f * h
                        clen = bps_f
                        nc.scalar.activation(a_sb[0:1, sub:sub + clen],
                                             a_sb[0:1, sub:sub + clen],
                                             mybir.ActivationFunctionType.Sigmoid,
                                             scale=1.0 / 64.0)
                        bps = bpsp.tile([128, bps_f], F32, tag="bps",
                                        name=f"bps{c}_{h}")
                        for j in range(bps_f // 512):
                            nc.tensor.matmul(
                                bps[:, 512 * j:512 * (j + 1)],
                                lhsT=ones_sb[:],
                                rhs=a_sb[0:1, sub + 512 * j:sub + 512 * (j + 1)],
                                start=True, stop=True,
                            )
                        bps_tiles.append(bps)
                    for d in range(nd):
                        vt = sp.tile([128, kv_f], F32, tag="kv", name=f"vt{c}_{d}")
                        nc.sync.dma_start(
                            vt[:], vc_d[128 * d:128 * (d + 1),
                                        kv_f * c:kv_f * (c + 1)])
                        for h in range(nh):
                            sc = scp.tile([128, bps_f], F32, tag="zsc",
                                          name=f"zs{c}_{h}_{d}")
                            acc = accp.tile([128, 1], F32, tag="acc",
                                            name=f"za{c}_{h}_{d}")
                            nc.vector._custom_dve(
                                TENSOR_TENSOR_REDUCE, out=sc[:],
                                in0=vt[:, bps_f * h:bps_f * (h + 1)],
                                in1=bps_tiles[h][:],
                                s0=0.0 if accs[d] is None else accs[d][:],
                                s1=1.0,
                                accum_out=acc[:],
                            )
                            accs[d] = acc

                # --- final column: z += v * a[t] ---
                nc.scalar.dma_start(a_sb[0:1, t:t + 8], cc_outs[2][:])
                nc.scalar.activation(a_sb[0:1, t:t + 1], a_sb[0:1, t:t + 1],
                                     mybir.ActivationFunctionType.Sigmoid,
                                     scale=1.0 / 64.0)
                a_last_b = kp.tile([128, 1], F32, tag="alb", name="a_last_b")
                nc.gpsimd.partition_broadcast(a_last_b[:], a_sb[0:1, t:t + 1])
                for d in range(nd):
                    sc1 = scp.tile([128, 1], F32, tag="sc1", name=f"zf{d}")
                    nc.vector._custom_dve(
                        TENSOR_TENSOR_REDUCE, out=sc1[:],
                        in0=qkv_all[:, 2 * nd + d:2 * nd + d + 1],
                        in1=a_last_b[:],
                        s0=accs[d][:], s1=1.0,
                        accum_out=z_final[:, d:d + 1],
                    )

                nc.gpsimd.dma_start(z_d[:], z_final[:])

    nc.compile()
    return nc


def make_in_maps(inputs, n_cores=N_CORES, d_sh=D // N_CORES):
    x = np.ascontiguousarray(np.asarray(inputs["x"], dtype=np.float32))
    in_maps = []
    for i in range(n_cores):
        r0, r1 = d_sh * i, d_sh * (i + 1)
        in_maps.append({
            "x": x,
            "wq": np.ascontiguousarray(np.asarray(inputs["W_query"])[r0:r1], np.float32),
            "wk": np.ascontiguousarray(np.asarray(inputs["W_key"])[r0:r1], np.float32),
            "wv": np.ascontiguousarray(np.asarray(inputs["W_value"])[r0:r1], np.float32),
            "kc": np.ascontiguousarray(np.asarray(inputs["K_cache"])[r0:r1], np.float32),
            "vc": np.ascontiguousarray(np.asarray(inputs["V_cache"])[r0:r1], np.float32),
        })
    return in_maps


def unshard(per_core_z, d_sh=D // N_CORES):
    shards = [np.asarray(zi).T.reshape(d_sh, 1) for zi in per_core_z]
    return np.concatenate(shards, axis=0).astype(np.float32)


_NC_CACHE = None


def kernel(x, W_query, W_key, W_value, K_cache, V_cache):
    global _NC_CACHE
    if _NC_CACHE is None:
        _NC_CACHE = build()
    nc = _NC_CACHE
    in_maps = make_in_maps(dict(x=x, W_query=W_query, W_key=W_key,
                                W_value=W_value, K_cache=K_cache,
                                V_cache=V_cache))
    res = run_bass_kernel_spmd(nc, in_maps, core_ids=list(range(N_CORES)))
    return unshard([res.results[i]["z"] for i in range(N_CORES)])



# revision 4
# speedup vs baseline: 1.1103x; 1.1103x over previous
"""Trainium2 Bass kernel for a sigmoid-scored attention decode step with KV cache.

Reference computation (all fp32):
    q = W_query @ x.T ; k = W_key @ x.T ; v = W_value @ x.T          # [4096, 1]
    K = [K_cache | k] ; V = [V_cache | v]                            # [4096, 8193]
    a = sigmoid((q.T @ K) / 64)                                      # [1, 8193]
    z = V @ a.T                                                      # [4096, 1]

Sharding: rows (output dim) of W_q/W_k/W_v/K_cache/V_cache are split across
8 NeuronCores (512 rows each). Each core computes its q/k/v shard and partial
scores over its 512 rows of K; AllReduces combine partials into full scores on
every core; sigmoid + the V-weighted sum are then local per shard. Host only
slices inputs and concatenates the output.

v2 schedule (vs the v1 baseline at 233us):
  - stream order Wq -> K(first half) -> K(second half) -> Wk -> Wv -> V so the
    score AllReduces fire at ~55/75us instead of ~100/155us; they complete
    during the Wv/V streaming instead of stalling the z-phase.
  - a tiny warm-up AllReduce (8 zeros, doubles as the s_sb pad) is issued at
    t~0 to absorb the cold-start cost of the first collective (~55us observed).
  - 4MB DMA transfers ([128, 8192] tiles covering 2 row-chunks x 4096 cols via
    an AP rearrange) instead of 2MB, single HWDGE ring, triple buffered.
  - the score broadcast for the z-phase is a gpsimd partition_broadcast into
    SBUF (off the critical path once ARs are early) instead of PE rank-1
    matmuls into PSUM, freeing PSUM and the PE tail.
"""

import sys

for _p in ("/opt/trn_rl_repo", "/root/.axon_site/_ro/trn_rl_repo"):
    if _p not in sys.path:
        sys.path.append(_p)

import numpy as np

import concourse.bacc as bacc
import concourse.tile as tile
from concourse import mybir
from concourse.bass_utils import run_bass_kernel_spmd
from concourse.dve_ops import TENSOR_TENSOR_REDUCE

N_CORES = 8
E = 4096          # embedding dim (contraction for q/k/v)
D = 4096          # output dim
T = 8192          # cached timesteps
F32 = mybir.dt.float32


def build(n_cores=N_CORES, e=E, d_sh=D // N_CORES, t=T):
    nd = d_sh // 128             # partition-chunks per core (4)
    half = t // 2                # score columns per AllReduce group (4096)
    RG = [list(range(n_cores))]

    nc = bacc.Bacc("TRN2", target_bir_lowering=False, debug=False,
                   num_devices=n_cores)
    x_d = nc.dram_tensor("x", [1, e], F32, kind="ExternalInput").ap()
    wq_d = nc.dram_tensor("wq", [d_sh, e], F32, kind="ExternalInput").ap()
    wk_d = nc.dram_tensor("wk", [d_sh, e], F32, kind="ExternalInput").ap()
    wv_d = nc.dram_tensor("wv", [d_sh, e], F32, kind="ExternalInput").ap()
    kc_d = nc.dram_tensor("kc", [d_sh, t], F32, kind="ExternalInput").ap()
    vc_d = nc.dram_tensor("vc", [d_sh, t], F32, kind="ExternalInput").ap()
    z_d = nc.dram_tensor("z", [128, nd], F32, kind="ExternalOutput").ap()

    def two_chunk(src):
        # [256, w] DRAM region -> [128, 2, w]: col block c holds rows
        # 128c..128c+127. Paired with a [p (c t) -> p c t] view of the tile.
        return src.rearrange("(c p) t -> p c t", p=128)

    def as3d(tile_ap, w):
        return tile_ap.rearrange("p (c t) -> p c t", t=w)

    with tile.TileContext(nc) as tc:
        with (
            tc.tile_pool(name="stream", bufs=3) as sp,       # [128, 8192] tiles
            tc.tile_pool(name="scratch", bufs=1) as scp,     # ttr elementwise outs
            tc.tile_pool(name="bcast", bufs=3) as bcp,       # [128, 2048] score bcasts
            tc.tile_pool(name="keep", bufs=1) as kp,         # persistent tiles
            tc.tile_pool(name="acc", bufs=8) as accp,        # [128,1] accumulators
            tc.tile_pool(name="dram", bufs=1, space="DRAM") as dramp,
        ):
            # --- warm-up collective: 8 zeros; output doubles as s_sb pad ---
            w_sb = kp.tile([1, 8], F32, tag="warmsb", name="w_sb")
            nc.vector.memset(w_sb[:], 0.0)
            cc_w_in = dramp.tile([1, 8], F32, tag="cc_w_in", name="cc_w_in")
            cc_w_out = dramp.tile([1, 8], F32, tag="cc_w_out", name="cc_w_out")
            nc.gpsimd.dma_start(cc_w_in[:], w_sb[:])
            nc.gpsimd.collective_compute(
                "AllReduce", mybir.AluOpType.add, replica_groups=RG,
                ins=[cc_w_in.opt()], outs=[cc_w_out.opt()],
            )

            # --- broadcast x across partitions ---
            x_sb = kp.tile([1, e], F32, tag="xsb", name="x_sb")
            nc.gpsimd.dma_start(x_sb[:], x_d[:])
            bx = kp.tile([128, e], F32, tag="bx", name="bx")
            nc.gpsimd.partition_broadcast(bx[:], x_sb[:])

            ones_sb = kp.tile([1, 128], F32, tag="ones", name="ones_sb")
            nc.vector.memset(ones_sb[:], 1.0)
            ones_col = kp.tile([128, 1], F32, tag="onesc", name="ones_col")
            nc.vector.memset(ones_col[:], 1.0)
            # pre-warm the sigmoid ACT table so the load is off the critical path
            warm = kp.tile([1, 1], F32, tag="warm", name="warm")
            nc.vector.memset(warm[:], 0.0)
            nc.scalar.activation(warm[:], warm[:],
                                 mybir.ActivationFunctionType.Sigmoid,
                                 scale=1.0 / 64.0)

            # scores staging: [1, t] partial/reduced scores + appended col + pad
            s_sb = kp.tile([1, t + 8], F32, tag="s", name="s_sb")
            # warm AR output = zeros land in the pad region t..t+8
            nc.scalar.dma_start(s_sb[0:1, t:t + 8], cc_w_out[:])

            qkv_all = kp.tile([128, 3 * nd], F32, tag="qkv", name="qkv_all")

            def w_matvec(w_dram, col0):
                # two [128, 2*e] transfers; TTR against bx per col block
                for k in range(2):
                    wt = sp.tile([128, 2 * e], F32, tag="big",
                                 name=f"wt{col0}_{k}")
                    nc.sync.dma_start(as3d(wt[:], e), two_chunk(w_dram[256 * k:256 * (k + 1), :]))
                    for c in range(2):
                        sc = scp.tile([128, e], F32, tag="sc",
                                      name=f"wsc{col0}_{k}_{c}")
                        nc.vector._custom_dve(
                            TENSOR_TENSOR_REDUCE, out=sc[:],
                            in0=wt[:, e * c:e * (c + 1)], in1=bx[:],
                            s0=0.0, s1=1.0,
                            accum_out=qkv_all[:, col0 + 2 * k + c:col0 + 2 * k + c + 1],
                        )

            w_matvec(wq_d, 0)        # q in cols 0..nd-1

            cc_ins = [dramp.tile([1, half], F32, tag=f"cc_in{g}",
                                 name=f"cc_in{g}") for g in range(2)]
            cc_outs = [dramp.tile([1, half], F32, tag=f"cc_out{g}",
                                  name=f"cc_out{g}") for g in range(2)]
            cc_in_l = dramp.tile([1, 8], F32, tag="cc_in_l", name="cc_in_l")
            cc_out_l = dramp.tile([1, 8], F32, tag="cc_out_l", name="cc_out_l")

            psp_ctx = tc.tile_pool(name="ps", bufs=1, space="PSUM")
            psp = psp_ctx.__enter__()

            def score_group(g):
                # partial scores for cols [half*g, half*(g+1)) over all nd
                # d-chunks; accumulate in one [1, half] PSUM tile (8 banks).
                ps = psp.tile([1, half], F32, tag="ps", name=f"ps{g}")
                for k in range(2):
                    kt = sp.tile([128, 2 * half], F32, tag="big",
                                 name=f"kt{g}_{k}")
                    nc.sync.dma_start(
                        as3d(kt[:], half),
                        two_chunk(kc_d[256 * k:256 * (k + 1),
                                       half * g:half * (g + 1)]))
                    for ci in range(2):
                        c = 2 * k + ci
                        for j in range(half // 512):
                            nc.tensor.matmul(
                                ps[0:1, 512 * j:512 * (j + 1)],
                                lhsT=qkv_all[:, c:c + 1],
                                rhs=kt[:, half * ci + 512 * j:half * ci + 512 * (j + 1)],
                                start=(c == 0), stop=(c == nd - 1),
                            )
                nc.vector.tensor_copy(s_sb[0:1, half * g:half * (g + 1)], ps[:])
                nc.gpsimd.dma_start(cc_ins[g][:], s_sb[0:1, half * g:half * (g + 1)])
                nc.gpsimd.collective_compute(
                    "AllReduce", mybir.AluOpType.add, replica_groups=RG,
                    ins=[cc_ins[g].opt()], outs=[cc_outs[g].opt()],
                )

            score_group(0)           # AR_A fires ~55us
            score_group(1)           # AR_B fires ~75us

            # --- k; appended-column partial score rides its own tiny AR ---
            w_matvec(wk_d, nd)       # k in cols nd..2nd-1
            qk_el = scp.tile([128, nd], F32, tag="sc", name="qk_el")
            qk_part = accp.tile([128, 1], F32, tag="acc", name="qk_part")
            nc.vector._custom_dve(
                TENSOR_TENSOR_REDUCE, out=qk_el[:], in0=qkv_all[:, 0:nd],
                in1=qkv_all[:, nd:2 * nd], s0=0.0, s1=1.0,
                accum_out=qk_part[:],
            )
            qk_ps = psp.tile([1, 512], F32, tag="ps", name="qk_ps")
            nc.tensor.matmul(qk_ps[0:1, 0:1], lhsT=ones_col[:],
                             rhs=qk_part[:], start=True, stop=True)
            nc.vector.tensor_copy(s_sb[0:1, t:t + 1], qk_ps[0:1, 0:1])
            nc.gpsimd.dma_start(cc_in_l[:], s_sb[0:1, t:t + 8])
            nc.gpsimd.collective_compute(
                "AllReduce", mybir.AluOpType.add, replica_groups=RG,
                ins=[cc_in_l.opt()], outs=[cc_out_l.opt()],
            )
            psp_ctx.__exit__(None, None, None)

            # --- score bcast tiles: load AR result, sigmoid, partition_bcast ---
            bcast = [[None, None], [None, None]]

            def make_bcast(g):
                nc.scalar.dma_start(s_sb[0:1, half * g:half * (g + 1)],
                                    cc_outs[g][:])
                for h in range(2):
                    lo = half * g + 2048 * h
                    nc.scalar.activation(s_sb[0:1, lo:lo + 2048],
                                         s_sb[0:1, lo:lo + 2048],
                                         mybir.ActivationFunctionType.Sigmoid,
                                         scale=1.0 / 64.0)
                    bt = bcp.tile([128, 2048], F32, tag="bc", name=f"bc{g}_{h}")
                    nc.gpsimd.partition_broadcast(bt[:], s_sb[0:1, lo:lo + 2048])
                    bcast[g][h] = bt

            make_bcast(0)
            w_matvec(wv_d, 2 * nd)   # v in cols 2nd..3nd-1
            make_bcast(1)

            # --- z accumulation: DVE reduce of V tiles against bcast tiles ---
            z_final = kp.tile([128, nd], F32, tag="z", name="z_final")
            accs = [None] * nd
            for g in range(2):
                for k in range(2):
                    vt = sp.tile([128, 2 * half], F32, tag="big",
                                 name=f"vt{g}_{k}")
                    nc.sync.dma_start(
                        as3d(vt[:], half),
                        two_chunk(vc_d[256 * k:256 * (k + 1),
                                       half * g:half * (g + 1)]))
                    for ci in range(2):
                        d = 2 * k + ci
                        for h in range(2):
                            sc = scp.tile([128, 2048], F32, tag="sc",
                                          name=f"zs{g}_{k}_{ci}_{h}")
                            acc = accp.tile([128, 1], F32, tag="acc",
                                            name=f"za{g}_{k}_{ci}_{h}")
                            nc.vector._custom_dve(
                                TENSOR_TENSOR_REDUCE, out=sc[:],
                                in0=vt[:, half * ci + 2048 * h:half * ci + 2048 * (h + 1)],
                                in1=bcast[g][h][:],
                                s0=0.0 if accs[d] is None else accs[d][:],
                                s1=1.0,
                                accum_out=acc[:],
                            )
                            accs[d] = acc

            # --- final column: z += v * a[t] ---
            a_f = kp.tile([1, 8], F32, tag="af", name="a_f")
            nc.scalar.dma_start(a_f[:], cc_out_l[:])
            nc.scalar.activation(a_f[0:1, 0:1], a_f[0:1, 0:1],
                                 mybir.ActivationFunctionType.Sigmoid,
                                 scale=1.0 / 64.0)
            with tc.tile_pool(name="ps2", bufs=1, space="PSUM") as psp2:
                a_last_b = psp2.tile([128, 1], F32, tag="alb", name="a_last_b")
                nc.tensor.matmul(a_last_b[:], lhsT=ones_sb[:],
                                 rhs=a_f[0:1, 0:1], start=True, stop=True)
                for d in range(nd):
                    sc1 = scp.tile([128, 1], F32, tag="sc", name=f"zf{d}")
                    nc.vector._custom_dve(
                        TENSOR_TENSOR_REDUCE, out=sc1[:],
                        in0=qkv_all[:, 2 * nd + d:2 * nd + d + 1],
                        in1=a_last_b[:],
                        s0=accs[d][:], s1=1.0,
                        accum_out=z_final[:, d:d + 1],
                    )

                nc.gpsimd.dma_start(z_d[:], z_final[:])

    nc.compile()
    return nc


def make_in_maps(inputs, n_cores=N_CORES, d_sh=D // N_CORES):
    x = np.ascontiguousarray(np.asarray(inputs["x"], dtype=np.float32))
    in_maps = []
    for i in range(n_cores):
        r0, r1 = d_sh * i, d_sh * (i + 1)
        in_maps.append({
            "x": x,
            "wq": np.ascontiguousarray(np.asarray(inputs["W_query"])[r0:r1], np.float32),
            "wk": np.ascontiguousarray(np.asarray(inputs["W_key"])[r0:r1], np.float32),
            "wv": np.ascontiguousarray(np.asarray(inputs["W_value"])[r0:r1], np.float32),
            "kc": np.ascontiguousarray(np.asarray(inputs["K_cache"])[r0:r1], np.float32),
            "vc": np.ascontiguousarray(np.asarray(inputs["V_cache"])[r0:r1], np.float32),
        })
    return in_maps


def unshard(per_core_z, d_sh=D // N_CORES):
    shards = [np.asarray(zi).T.reshape(d_sh, 1) for zi in per_core_z]
    return np.concatenate(shards, axis=0).astype(np.float32)


_NC_CACHE = None


def kernel(x, W_query, W_key, W_value, K_cache, V_cache):
    global _NC_CACHE
    if _NC_CACHE is None:
        _NC_CACHE = build()
    nc = _NC_CACHE
    in_maps = make_in_maps(dict(x=x, W_query=W_query, W_key=W_key,
                                W_value=W_value, K_cache=K_cache,
                                V_cache=V_cache))
    res = run_bass_kernel_spmd(nc, in_maps, core_ids=list(range(N_CORES)))
    return unshard([res.results[i]["z"] for i in range(N_CORES)])


# revision 5
# speedup vs baseline: 1.1617x; 1.0463x over previous
"""Trainium2 Bass kernel for a sigmoid-scored attention decode step with KV cache.

Reference computation (all fp32):
    q = W_query @ x.T ; k = W_key @ x.T ; v = W_value @ x.T          # [4096, 1]
    K = [K_cache | k] ; V = [V_cache | v]                            # [4096, 8193]
    a = sigmoid((q.T @ K) / 64)                                      # [1, 8193]
    z = V @ a.T                                                      # [4096, 1]

Sharding: rows (output dim) of W_q/W_k/W_v/K_cache/V_cache are split across
8 NeuronCores (512 rows each). Each core computes its q/k/v shard and partial
scores over its 512 rows of K; one AllReduce combines partials into full
scores on every core; sigmoid + the V-weighted sum are then local per shard.
Host slices inputs, casts them to bf16 (the correctness gate is 2e-2 and the
problem is HBM-bandwidth-bound, so halving traffic is the dominant win), and
concatenates the output.

v3 schedule (vs v2 at 210us, v1 at 233us):
  - all inputs cast to bf16 on host: 28MB/core HBM traffic instead of 56MB.
  - ONE merged AllReduce (both score halves + appended-column partial, bf16)
    instead of three serial f32 ones: the collective pipeline on this runtime
    costs ~28us per op serially, after a fixed ~53us init, so fewer is better.
  - stream order Wq -> K -> Wk -> Wv -> V; the AR fires at ~60us and lands
    while V is still streaming.
  - z-phase: sigmoid (in 2048-chunks) -> gpsimd partition_broadcast -> DVE
    TENSOR_TENSOR_REDUCE per [128,2048] V slice, chained per row-chunk.
"""

import sys

for _p in ("/opt/trn_rl_repo", "/root/.axon_site/_ro/trn_rl_repo"):
    if _p not in sys.path:
        sys.path.append(_p)

import ml_dtypes
import numpy as np

import concourse.bacc as bacc
import concourse.tile as tile
from concourse import mybir
from concourse.bass_utils import run_bass_kernel_spmd
from concourse.dve_ops import TENSOR_TENSOR_REDUCE

N_CORES = 8
E = 4096          # embedding dim (contraction for q/k/v)
D = 4096          # output dim
T = 8192          # cached timesteps
F32 = mybir.dt.float32
BF16 = mybir.dt.bfloat16
BF16_NP = ml_dtypes.bfloat16


def build(n_cores=N_CORES, e=E, d_sh=D // N_CORES, t=T):
    nd = d_sh // 128             # partition-chunks per core (4)
    half = t // 2                # score columns per K column-group (4096)
    pay = t + 16                 # AllReduce payload cols (scores + qk + pad)
    RG = [list(range(n_cores))]

    nc = bacc.Bacc("TRN2", target_bir_lowering=False, debug=False,
                   num_devices=n_cores)
    x_d = nc.dram_tensor("x", [1, e], BF16, kind="ExternalInput").ap()
    wq_d = nc.dram_tensor("wq", [d_sh, e], BF16, kind="ExternalInput").ap()
    wk_d = nc.dram_tensor("wk", [d_sh, e], BF16, kind="ExternalInput").ap()
    wv_d = nc.dram_tensor("wv", [d_sh, e], BF16, kind="ExternalInput").ap()
    kc_d = nc.dram_tensor("kc", [d_sh, t], BF16, kind="ExternalInput").ap()
    vc_d = nc.dram_tensor("vc", [d_sh, t], BF16, kind="ExternalInput").ap()
    z_d = nc.dram_tensor("z", [128, nd], F32, kind="ExternalOutput").ap()

    def chunked(src, nchunk):
        # [nchunk*128, w] DRAM region -> [128, nchunk, w]: block c holds rows
        # 128c..128c+127. Paired with a [p (c t) -> p c t] view of the tile.
        return src.rearrange("(c p) t -> p c t", p=128)

    def as3d(tile_ap, w):
        return tile_ap.rearrange("p (c t) -> p c t", t=w)

    with tile.TileContext(nc) as tc:
        with (
            tc.tile_pool(name="stream", bufs=4) as sp,       # streamed W/K/V tiles
            tc.tile_pool(name="scratch", bufs=1) as scp,     # ttr elementwise outs
            tc.tile_pool(name="bcast", bufs=4) as bcp,       # [128, 2048] score bcasts
            tc.tile_pool(name="keep", bufs=1) as kp,         # persistent tiles
            tc.tile_pool(name="acc", bufs=8) as accp,        # [128,1] accumulators
            tc.tile_pool(name="dram", bufs=1, space="DRAM") as dramp,
        ):
            # --- broadcast x across partitions ---
            x_sb = kp.tile([1, e], BF16, tag="xsb", name="x_sb")
            nc.gpsimd.dma_start(x_sb[:], x_d[:])
            bx = kp.tile([128, e], BF16, tag="bx", name="bx")
            nc.gpsimd.partition_broadcast(bx[:], x_sb[:])

            ones_sb = kp.tile([1, 128], BF16, tag="ones", name="ones_sb")
            nc.vector.memset(ones_sb[:], 1.0)
            ones_col = kp.tile([128, 1], BF16, tag="onesc", name="ones_col")
            nc.vector.memset(ones_col[:], 1.0)
            # pre-warm the sigmoid ACT table so the load is off the critical path
            warm = kp.tile([1, 1], BF16, tag="warm", name="warm")
            nc.vector.memset(warm[:], 0.0)
            nc.scalar.activation(warm[:], warm[:],
                                 mybir.ActivationFunctionType.Sigmoid,
                                 scale=1.0 / 64.0)

            # partial scores staging (bf16): [0,t) cache scores, t = appended
            # column, (t, t+16) zero pad
            s_sb = kp.tile([1, pay], BF16, tag="s", name="s_sb")
            nc.vector.memset(s_sb[0:1, t + 1:pay], 0.0)

            qkv_all = kp.tile([128, 3 * nd], BF16, tag="qkv", name="qkv_all")

            def w_matvec(w_dram, col0):
                # one [128, 4*e] transfer; TTR against bx per col block
                wt = sp.tile([128, nd * e], BF16, tag="big", name=f"wt{col0}")
                nc.sync.dma_start(as3d(wt[:], e), chunked(w_dram, nd))
                for c in range(nd):
                    sc = scp.tile([128, e], BF16, tag="sc",
                                  name=f"wsc{col0}_{c}")
                    nc.vector._custom_dve(
                        TENSOR_TENSOR_REDUCE, out=sc[:],
                        in0=wt[:, e * c:e * (c + 1)], in1=bx[:],
                        s0=0.0, s1=1.0,
                        accum_out=qkv_all[:, col0 + c:col0 + c + 1],
                    )

            w_matvec(wq_d, 0)        # q in cols 0..nd-1

            cc_in = dramp.tile([1, pay], BF16, tag="cc_in", name="cc_in")
            cc_out = dramp.tile([1, pay], BF16, tag="cc_out", name="cc_out")

            psp_ctx = tc.tile_pool(name="ps", bufs=1, space="PSUM")
            psp = psp_ctx.__enter__()

            def score_group(g):
                # partial scores for cols [half*g, half*(g+1)) over all nd
                # d-chunks; accumulate in one [1, half] PSUM tile (8 banks).
                # two [128, 2*half] transfers (2 d-chunks each).
                ps = psp.tile([1, half], F32, tag="ps", name=f"ps{g}")
                for k in range(2):
                    kt = sp.tile([128, 2 * half], BF16, tag="big",
                                 name=f"kt{g}_{k}")
                    nc.sync.dma_start(
                        as3d(kt[:], half),
                        chunked(kc_d[256 * k:256 * (k + 1),
                                     half * g:half * (g + 1)], 2))
                    for ci in range(2):
                        c = 2 * k + ci
                        for j in range(half // 512):
                            nc.tensor.matmul(
                                ps[0:1, 512 * j:512 * (j + 1)],
                                lhsT=qkv_all[:, c:c + 1],
                                rhs=kt[:, half * ci + 512 * j:half * ci + 512 * (j + 1)],
                                start=(c == 0), stop=(c == nd - 1),
                            )
                # copy-cast f32 PSUM -> bf16 staging
                nc.vector.tensor_copy(s_sb[0:1, half * g:half * (g + 1)], ps[:])

            score_group(0)
            score_group(1)

            # --- k; appended-column partial score q.k ---
            w_matvec(wk_d, nd)       # k in cols nd..2nd-1
            qk_el = scp.tile([128, nd], BF16, tag="sc", name="qk_el")
            qk_part = accp.tile([128, 1], BF16, tag="acc", name="qk_part")
            nc.vector._custom_dve(
                TENSOR_TENSOR_REDUCE, out=qk_el[:], in0=qkv_all[:, 0:nd],
                in1=qkv_all[:, nd:2 * nd], s0=0.0, s1=1.0,
                accum_out=qk_part[:],
            )
            qk_ps = psp.tile([1, 512], F32, tag="ps", name="qk_ps")
            nc.tensor.matmul(qk_ps[0:1, 0:1], lhsT=ones_col[:],
                             rhs=qk_part[:], start=True, stop=True)
            nc.vector.tensor_copy(s_sb[0:1, t:t + 1], qk_ps[0:1, 0:1])
            psp_ctx.__exit__(None, None, None)

            # --- ONE AllReduce for everything ---
            nc.gpsimd.dma_start(cc_in[:], s_sb[:])
            nc.gpsimd.collective_compute(
                "AllReduce", mybir.AluOpType.add, replica_groups=RG,
                ins=[cc_in.opt()], outs=[cc_out.opt()],
            )

            w_matvec(wv_d, 2 * nd)   # v in cols 2nd..3nd-1

            # --- reduced scores: load, sigmoid in 2048-chunks, partition bcast
            nc.scalar.dma_start(s_sb[:], cc_out[:])
            bcast = []
            for h in range(4):
                lo = 2048 * h
                nc.scalar.activation(s_sb[0:1, lo:lo + 2048],
                                     s_sb[0:1, lo:lo + 2048],
                                     mybir.ActivationFunctionType.Sigmoid,
                                     scale=1.0 / 64.0)
                bt = bcp.tile([128, 2048], BF16, tag="bc", name=f"bc{h}")
                nc.gpsimd.partition_broadcast(bt[:], s_sb[0:1, lo:lo + 2048])
                bcast.append(bt)
            nc.scalar.activation(s_sb[0:1, t:t + 1], s_sb[0:1, t:t + 1],
                                 mybir.ActivationFunctionType.Sigmoid,
                                 scale=1.0 / 64.0)

            # --- z accumulation: DVE reduce of V tiles against bcast tiles ---
            z_final = kp.tile([128, nd], F32, tag="z", name="z_final")
            accs = [None] * nd
            for g in range(2):
                for k in range(2):
                    vt = sp.tile([128, 2 * half], BF16, tag="big",
                                 name=f"vt{g}_{k}")
                    nc.sync.dma_start(
                        as3d(vt[:], half),
                        chunked(vc_d[256 * k:256 * (k + 1),
                                     half * g:half * (g + 1)], 2))
                    for ci in range(2):
                        d = 2 * k + ci
                        for h in range(2):
                            sc = scp.tile([128, 2048], BF16, tag="sc",
                                          name=f"zs{g}_{k}_{ci}_{h}")
                            acc = accp.tile([128, 1], F32, tag="acc",
                                            name=f"za{g}_{k}_{ci}_{h}")
                            nc.vector._custom_dve(
                                TENSOR_TENSOR_REDUCE, out=sc[:],
                                in0=vt[:, half * ci + 2048 * h:half * ci + 2048 * (h + 1)],
                                in1=bcast[2 * g + h][:],
                                s0=0.0 if accs[d] is None else accs[d][:],
                                s1=1.0,
                                accum_out=acc[:],
                            )
                            accs[d] = acc

            # --- final column: z += v * a[t] ---
            with tc.tile_pool(name="ps2", bufs=1, space="PSUM") as psp2:
                a_last_ps = psp2.tile([128, 1], F32, tag="alb", name="a_last_ps")
                nc.tensor.matmul(a_last_ps[:], lhsT=ones_sb[:],
                                 rhs=s_sb[0:1, t:t + 1], start=True, stop=True)
                a_last_sb = kp.tile([128, 1], BF16, tag="als", name="a_last_sb")
                nc.vector.tensor_copy(a_last_sb[:], a_last_ps[:])
                for d in range(nd):
                    sc1 = scp.tile([128, 1], BF16, tag="sc", name=f"zf{d}")
                    nc.vector._custom_dve(
                        TENSOR_TENSOR_REDUCE, out=sc1[:],
                        in0=qkv_all[:, 2 * nd + d:2 * nd + d + 1],
                        in1=a_last_sb[:],
                        s0=accs[d][:], s1=1.0,
                        accum_out=z_final[:, d:d + 1],
                    )

                nc.gpsimd.dma_start(z_d[:], z_final[:])

    nc.compile()
    return nc


def make_in_maps(inputs, n_cores=N_CORES, d_sh=D // N_CORES):
    def bf(a):
        return np.ascontiguousarray(np.asarray(a, np.float32).astype(BF16_NP))

    x = bf(inputs["x"])
    wq = bf(inputs["W_query"])
    wk = bf(inputs["W_key"])
    wv = bf(inputs["W_value"])
    kc = bf(inputs["K_cache"])
    vc = bf(inputs["V_cache"])
    in_maps = []
    for i in range(n_cores):
        r0, r1 = d_sh * i, d_sh * (i + 1)
        in_maps.append({
            "x": x,
            "wq": np.ascontiguousarray(wq[r0:r1]),
            "wk": np.ascontiguousarray(wk[r0:r1]),
            "wv": np.ascontiguousarray(wv[r0:r1]),
            "kc": np.ascontiguousarray(kc[r0:r1]),
            "vc": np.ascontiguousarray(vc[r0:r1]),
        })
    return in_maps


def unshard(per_core_z, d_sh=D // N_CORES):
    shards = [np.asarray(zi).T.reshape(d_sh, 1) for zi in per_core_z]
    return np.concatenate(shards, axis=0).astype(np.float32)


_NC_CACHE = None


def kernel(x, W_query, W_key, W_value, K_cache, V_cache):
    global _NC_CACHE
    if _NC_CACHE is None:
        _NC_CACHE = build()
    nc = _NC_CACHE
    in_maps = make_in_maps(dict(x=x, W_query=W_query, W_key=W_key,
                                W_value=W_value, K_cache=K_cache,
                                V_cache=V_cache))
    res = run_bass_kernel_spmd(nc, in_maps, core_ids=list(range(N_CORES)))
    return unshard([res.results[i]["z"] for i in range(N_CORES)])


# revision 7
# speedup vs baseline: 1.3054x; 1.1237x over previous
"""Trainium2 Bass kernel for a sigmoid-scored attention decode step with KV cache.

Reference computation (all fp32):
    q = W_query @ x.T ; k = W_key @ x.T ; v = W_value @ x.T          # [4096, 1]
    K = [K_cache | k] ; V = [V_cache | v]                            # [4096, 8193]
    a = sigmoid((q.T @ K) / 64)                                      # [1, 8193]
    z = V @ a.T                                                      # [4096, 1]

Sharding: rows (output dim) of W_q/W_k/W_v/K_cache/V_cache are split across
8 NeuronCores (512 rows each). Each core computes its q/k/v shard and partial
scores over its 512 rows of K; one AllReduce combines partials into full
scores on every core; sigmoid + the V-weighted sum are then local per shard.
Host slices inputs, casts them to bf16 (the correctness gate is 2e-2 and the
problem is HBM-bandwidth-bound, so halving traffic is the dominant win), and
pre-transposes the V shard so the z-phase runs on the tensor engine.

v4 schedule (vs v3 200us / v2 210us / v1 233us):
  - z = V @ a on the (otherwise idle) PE instead of DVE: host supplies V^T
    [8192, 512]; a is loaded in an interleaved [128, 64] layout via one xbar
    transpose-DMA of the AllReduce output, so sigmoid is a single [128, 64]
    ACT op. This removes ~47us of serial DVE reduce + ~18us of gpsimd
    partition_broadcast from the post-AllReduce tail.
  - x is broadcast by a stride-0 DMA read instead of gpsimd
    partition_broadcast (which cost 6us and delayed the first TTR).
  - z comes out as a row vector [1, 512] (host just reshapes).
"""

import sys

for _p in ("/opt/trn_rl_repo", "/root/.axon_site/_ro/trn_rl_repo"):
    if _p not in sys.path:
        sys.path.append(_p)

import ml_dtypes
import numpy as np

import concourse.bacc as bacc
import concourse.tile as tile
from concourse import mybir
from concourse.bass_utils import run_bass_kernel_spmd
from concourse.dve_ops import TENSOR_TENSOR_REDUCE

N_CORES = 8
E = 4096          # embedding dim (contraction for q/k/v)
D = 4096          # output dim
T = 8192          # cached timesteps
F32 = mybir.dt.float32
BF16 = mybir.dt.bfloat16
BF16_NP = ml_dtypes.bfloat16


def build(n_cores=N_CORES, e=E, d_sh=D // N_CORES, t=T):
    nd = d_sh // 128             # partition-chunks per core (4)
    half = t // 2                # score columns per K column-group (4096)
    pay = t + 16                 # AllReduce payload cols (scores + qk + pad)
    nr = t // 128                # t-chunks for the PE z phase (64)
    RG = [list(range(n_cores))]

    nc = bacc.Bacc("TRN2", target_bir_lowering=False, debug=False,
                   num_devices=n_cores)
    x_d = nc.dram_tensor("x", [1, e], BF16, kind="ExternalInput").ap()
    wq_d = nc.dram_tensor("wq", [d_sh, e], BF16, kind="ExternalInput").ap()
    wk_d = nc.dram_tensor("wk", [d_sh, e], BF16, kind="ExternalInput").ap()
    wv_d = nc.dram_tensor("wv", [d_sh, e], BF16, kind="ExternalInput").ap()
    kc_d = nc.dram_tensor("kc", [d_sh, t], BF16, kind="ExternalInput").ap()
    vct_d = nc.dram_tensor("vct", [t, d_sh], BF16, kind="ExternalInput").ap()
    z_d = nc.dram_tensor("z", [1, d_sh], F32, kind="ExternalOutput").ap()

    def chunked(src):
        # [n*128, w] DRAM region -> [128, n, w]: block c holds rows
        # 128c..128c+127. Paired with a [p (c t) -> p c t] view of the tile.
        return src.rearrange("(c p) t -> p c t", p=128)

    def as3d(tile_ap, w):
        return tile_ap.rearrange("p (c t) -> p c t", t=w)

    with tile.TileContext(nc) as tc:
        with (
            tc.tile_pool(name="stream", bufs=6) as sp,       # streamed 2MB tiles
            tc.tile_pool(name="scratch", bufs=1) as scp,     # ttr elementwise outs
            tc.tile_pool(name="keep", bufs=1) as kp,         # persistent tiles
            tc.tile_pool(name="acc", bufs=8) as accp,        # [128,1] accumulators
            tc.tile_pool(name="dram", bufs=1, space="DRAM") as dramp,
        ):
            # --- x broadcast across partitions via stride-0 DMA read ---
            bx = kp.tile([128, e], BF16, tag="bx", name="bx")
            nc.sync.dma_start(bx[:], x_d[0:1, :].partition_broadcast(128))

            ones_col = kp.tile([128, 1], BF16, tag="onesc", name="ones_col")
            nc.vector.memset(ones_col[:], 1.0)
            # pre-warm the sigmoid ACT table so the load is off the critical path
            warm = kp.tile([1, 1], BF16, tag="warm", name="warm")
            nc.vector.memset(warm[:], 0.0)
            nc.scalar.activation(warm[:], warm[:],
                                 mybir.ActivationFunctionType.Sigmoid,
                                 scale=1.0 / 64.0)

            # partial scores staging (bf16): [0,t) cache scores, t = appended
            # column, (t, t+16) zero pad
            s_sb = kp.tile([1, pay], BF16, tag="s", name="s_sb")
            nc.vector.memset(s_sb[0:1, t + 1:pay], 0.0)

            qkv_all = kp.tile([128, 3 * nd], BF16, tag="qkv", name="qkv_all")

            def w_matvec(w_dram, col0):
                # two [128, 2*e] transfers; TTR against bx per col block
                for k in range(2):
                    wt = sp.tile([128, 2 * e], BF16, tag="big",
                                 name=f"wt{col0}_{k}")
                    nc.sync.dma_start(as3d(wt[:], e),
                                      chunked(w_dram[256 * k:256 * (k + 1), :]))
                    for c in range(2):
                        sc = scp.tile([128, e], BF16, tag="sc",
                                      name=f"wsc{col0}_{k}_{c}")
                        nc.vector._custom_dve(
                            TENSOR_TENSOR_REDUCE, out=sc[:],
                            in0=wt[:, e * c:e * (c + 1)], in1=bx[:],
                            s0=0.0, s1=1.0,
                            accum_out=qkv_all[:, col0 + 2 * k + c:col0 + 2 * k + c + 1],
                        )

            w_matvec(wq_d, 0)        # q in cols 0..nd-1
            w_matvec(wk_d, nd)       # k in cols nd..2nd-1

            cc_in = dramp.tile([1, pay], BF16, tag="cc_in", name="cc_in")
            cc_out = dramp.tile([1, pay], BF16, tag="cc_out", name="cc_out")

            psq_ctx = tc.tile_pool(name="psq", bufs=1, space="PSUM")
            psq = psq_ctx.__enter__()
            psp_ctx = tc.tile_pool(name="ps", bufs=3, space="PSUM")
            psp = psp_ctx.__enter__()

            # --- partial scores: [1,1024] PSUM tiles, 4 per column group ---
            score_tiles = {}
            for g in range(2):
                for k in range(2):
                    kt = sp.tile([128, 2 * half], BF16, tag="big",
                                 name=f"kt{g}_{k}")
                    nc.sync.dma_start(
                        as3d(kt[:], half),
                        chunked(kc_d[256 * k:256 * (k + 1),
                                     half * g:half * (g + 1)]))
                    for ci in range(2):
                        c = 2 * k + ci
                        for i in range(4):
                            if k == 0 and ci == 0:
                                score_tiles[(g, i)] = psp.tile(
                                    [1, 1024], F32, tag="ps", name=f"ps{g}_{i}")
                            ps = score_tiles[(g, i)]
                            for j in range(2):
                                lo = half * ci + 1024 * i + 512 * j
                                nc.tensor.matmul(
                                    ps[0:1, 512 * j:512 * (j + 1)],
                                    lhsT=qkv_all[:, c:c + 1],
                                    rhs=kt[:, lo:lo + 512],
                                    start=(c == 0), stop=(c == nd - 1),
                                )
                # copy-cast f32 PSUM -> bf16 staging
                for i in range(4):
                    nc.vector.tensor_copy(
                        s_sb[0:1, half * g + 1024 * i:half * g + 1024 * (i + 1)],
                        score_tiles[(g, i)][:])

            # --- appended-column partial score q.k ---
            qk_el = scp.tile([128, nd], BF16, tag="sc", name="qk_el")
            qk_part = accp.tile([128, 1], BF16, tag="acc", name="qk_part")
            nc.vector._custom_dve(
                TENSOR_TENSOR_REDUCE, out=qk_el[:], in0=qkv_all[:, 0:nd],
                in1=qkv_all[:, nd:2 * nd], s0=0.0, s1=1.0,
                accum_out=qk_part[:],
            )
            qk_ps = psq.tile([1, 512], F32, tag="psq", name="qk_ps")
            nc.tensor.matmul(qk_ps[0:1, 0:1], lhsT=ones_col[:],
                             rhs=qk_part[:], start=True, stop=True)
            nc.vector.tensor_copy(s_sb[0:1, t:t + 1], qk_ps[0:1, 0:1])

            # --- ONE AllReduce for everything ---
            nc.gpsimd.dma_start(cc_in[:], s_sb[:])
            nc.gpsimd.collective_compute(
                "AllReduce", mybir.AluOpType.add, replica_groups=RG,
                ins=[cc_in.opt()], outs=[cc_out.opt()],
            )

            w_matvec(wv_d, 2 * nd)   # v in cols 2nd..3nd-1

            # v as a [1, 512] row for the appended-column matmul: bounce the
            # blocked qkv columns through DRAM in transposed order (tiny, off
            # the critical path)
            vrow_d = dramp.tile([nd, 128], BF16, tag="vrow", name="vrow_d")
            nc.gpsimd.dma_start(vrow_d.rearrange("c p -> p c"),
                                qkv_all[:, 2 * nd:3 * nd])
            v_row = kp.tile([1, d_sh], BF16, tag="vrow_sb", name="v_row")
            nc.scalar.dma_start(v_row[:], vrow_d[:].flatten().unsqueeze(0))

            psp_ctx.__exit__(None, None, None)
            psq_ctx.__exit__(None, None, None)

            # --- reduced scores -> interleaved [128, 64] via xbar transpose;
            # --- sigmoid is then one cheap [128, 64] ACT op
            a_t = kp.tile([128, nr], BF16, tag="at", name="a_t")
            nc.sync.dma_start_transpose(
                a_t[:], cc_out[0:1, 0:t].rearrange("1 (r p) -> r p", p=128))
            nc.scalar.activation(a_t[:], a_t[:],
                                 mybir.ActivationFunctionType.Sigmoid,
                                 scale=1.0 / 64.0)
            a_f = kp.tile([1, 8], BF16, tag="af", name="a_f")
            nc.scalar.dma_start(a_f[:], cc_out[0:1, t:t + 8])
            nc.scalar.activation(a_f[0:1, 0:1], a_f[0:1, 0:1],
                                 mybir.ActivationFunctionType.Sigmoid,
                                 scale=1.0 / 64.0)

            # --- z = V @ a on PE: 64 accumulating matmuls + appended column ---
            with tc.tile_pool(name="ps2", bufs=1, space="PSUM") as psp2:
                z_ps = psp2.tile([1, d_sh], F32, tag="zps", name="z_ps")
                for k in range(4):
                    vt = sp.tile([128, 16 * d_sh], BF16, tag="big",
                                 name=f"vt{k}")
                    nc.sync.dma_start(
                        as3d(vt[:], d_sh),
                        chunked(vct_d[2048 * k:2048 * (k + 1), :]))
                    for r in range(16):
                        nc.tensor.matmul(
                            z_ps[:], lhsT=a_t[:, 16 * k + r:16 * k + r + 1],
                            rhs=vt[:, 512 * r:512 * (r + 1)],
                            start=(k == 0 and r == 0), stop=False,
                        )
                nc.tensor.matmul(z_ps[:], lhsT=a_f[0:1, 0:1], rhs=v_row[:],
                                 start=False, stop=True)

                z_sb = kp.tile([1, d_sh], F32, tag="zsb", name="z_sb")
                nc.vector.tensor_copy(z_sb[:], z_ps[:])
                nc.gpsimd.dma_start(z_d[:], z_sb[:])

    nc.compile()
    return nc


def make_in_maps(inputs, n_cores=N_CORES, d_sh=D // N_CORES):
    def bf(a):
        return np.ascontiguousarray(np.asarray(a, np.float32).astype(BF16_NP))

    x = bf(inputs["x"])
    wq = bf(inputs["W_query"])
    wk = bf(inputs["W_key"])
    wv = bf(inputs["W_value"])
    kc = bf(inputs["K_cache"])
    vc = bf(inputs["V_cache"])
    in_maps = []
    for i in range(n_cores):
        r0, r1 = d_sh * i, d_sh * (i + 1)
        in_maps.append({
            "x": x,
            "wq": np.ascontiguousarray(wq[r0:r1]),
            "wk": np.ascontiguousarray(wk[r0:r1]),
            "wv": np.ascontiguousarray(wv[r0:r1]),
            "kc": np.ascontiguousarray(kc[r0:r1]),
            "vct": np.ascontiguousarray(vc[r0:r1].T),
        })
    return in_maps


def unshard(per_core_z, d_sh=D // N_CORES):
    shards = [np.asarray(zi).reshape(d_sh, 1) for zi in per_core_z]
    return np.concatenate(shards, axis=0).astype(np.float32)


_NC_CACHE = None


def kernel(x, W_query, W_key, W_value, K_cache, V_cache):
    global _NC_CACHE
    if _NC_CACHE is None:
        _NC_CACHE = build()
    nc = _NC_CACHE
    in_maps = make_in_maps(dict(x=x, W_query=W_query, W_key=W_key,
                                W_value=W_value, K_cache=K_cache,
                                V_cache=V_cache))
    res = run_bass_kernel_spmd(nc, in_maps, core_ids=list(range(N_CORES)))
    return unshard([res.results[i]["z"] for i in range(N_CORES)])
